# revision 3
# baseline (speedup 1.0000x reference)
"""Trainium2 Bass kernel for clamped cubic B-spline basis evaluation.

Computes, for x: [N] f32 and a clamped knot vector t (K=10, degree 3):
    z = (x - min(x)) / (max(x) - min(x) + 1e-8)
    out[n, j] = B_j^3(z[n]),  j = 0..5   -> [N, 6] f32

Strategy: trivially data-parallel over 8 NeuronCores (N/8 points each).
Per core, points are processed in [128 x FD] tiles.  The Cox-de Boor
recursion is evaluated with a continuous reformulation (relu/min hat
functions; every basis blend is two scalar_tensor_tensor products plus
an add), which matches the reference's masked recursion to ~1e-6 because
all selection boundaries are continuous crossings.

The [N, 6] output is assembled interleaved in SBUF (stride-6 writes by
the final blend ops) so the DRAM store is a single contiguous DMA.
"""

import numpy as np

N_POINTS = 8_388_608
N_CORES = 8
P = 128          # SBUF partitions
FD = 512         # free-dim elements per tile
N_SHARD = N_POINTS // N_CORES
TILE_ELEMS = P * FD
T_TILES = N_SHARD // TILE_ELEMS

_cache = {}


def _build(c1, c2):
    """Build + compile the per-core Bass program. c1, c2: interior knots."""
    import concourse.bacc as bacc
    import concourse.mybir as mybir
    import concourse.tile as tile

    f32 = mybir.dt.float32
    AF = mybir.ActivationFunctionType
    ALU = mybir.AluOpType

    nc = bacc.Bacc("TRN2", target_bir_lowering=False, debug=False)
    x_d = nc.dram_tensor("x", [T_TILES, P, FD], f32, kind="ExternalInput")
    st_d = nc.dram_tensor("stats", [P, 4], f32, kind="ExternalInput")
    o_d = nc.dram_tensor("out", [T_TILES, P, FD * 6], f32, kind="ExternalOutput")
    x_ap, st_ap, o_ap = x_d.ap(), st_d.ap(), o_d.ap()

    # knot-derived compile-time constants
    rc1 = 1.0 / c1                    # 1/c1
    rc2 = 1.0 / c2
    rdc = 1.0 / (c2 - c1)
    rg1 = 1.0 / (1.0 - c1)
    rg2 = 1.0 / (1.0 - c2)

    with tile.TileContext(nc) as tc:
        with (
            tc.tile_pool(name="io", bufs=3) as io,
            tc.tile_pool(name="wk", bufs=2) as wk,
            tc.tile_pool(name="cst", bufs=1) as cst,
        ):
            st = cst.tile([P, 4], f32, tag="st")
            nc.sync.dma_start(st[:], st_ap[:])
            s_ap = st[:, 0:1]
            b_ap = st[:, 1:2]
            b15_ap = st[:, 2:3]

            for t in range(T_TILES):
                xt = io.tile([P, FD], f32, tag="x")
                nc.sync.dma_start(xt[:], x_ap[t])

                z = wk.tile([P, FD], f32, tag="z")
                # z >= 0 by construction, so Relu == identity here (and Copy
                # does not accept an AP bias).
                nc.scalar.activation(z[:], xt[:], AF.Relu, bias=b_ap, scale=s_ap)

                # level-1 corner chains + their squares (ACT)
                b12 = wk.tile([P, FD], f32, tag="b12")
                nc.scalar.activation(b12[:], z[:], AF.Relu, bias=1.0, scale=-rc1)
                b15 = wk.tile([P, FD], f32, tag="b15")
                nc.scalar.activation(b15[:], z[:], AF.Relu, bias=b15_ap, scale=rg2)
                b21 = wk.tile([P, FD], f32, tag="b21")
                nc.scalar.activation(b21[:], b12[:], AF.Square)
                b25 = wk.tile([P, FD], f32, tag="b25")
                nc.scalar.activation(b25[:], b15[:], AF.Square)

                # weight tensors (gpsimd, 1-input line rate)
                wG = wk.tile([P, FD], f32, tag="wG")   # 1 - z
                nc.gpsimd.tensor_scalar(wG[:], z[:], -1.0, 1.0, ALU.mult, ALU.add)
                wM = wk.tile([P, FD], f32, tag="wM")   # c2 - z
                nc.gpsimd.tensor_scalar(wM[:], z[:], -1.0, c2, ALU.mult, ALU.add)
                wh = wk.tile([P, FD], f32, tag="wh")   # z - c1
                nc.gpsimd.tensor_scalar(wh[:], z[:], 1.0, -c1, ALU.mult, ALU.add)
                we = wk.tile([P, FD], f32, tag="we")   # z - c2
                nc.gpsimd.tensor_scalar(we[:], z[:], 1.0, -c2, ALU.mult, ALU.add)

                # hats (DVE)
                s1 = wk.tile([P, FD], f32, tag="s1")
                nc.vector.tensor_scalar(s1[:], z[:], -rdc, c2 * rdc, ALU.mult, ALU.add)
                s2 = wk.tile([P, FD], f32, tag="s2")
                nc.vector.tensor_scalar_mul(s2[:], z[:], rc1)
                b13 = wk.tile([P, FD], f32, tag="b13")
                nc.vector.tensor_tensor(b13[:], s2[:], s1[:], ALU.min)
                nc.vector.tensor_scalar_max(b13[:], b13[:], 0.0)

                s3 = wk.tile([P, FD], f32, tag="s3")
                nc.vector.tensor_scalar_mul(s3[:], wh[:], rdc)
                s4 = wk.tile([P, FD], f32, tag="s4")
                nc.vector.tensor_scalar_mul(s4[:], wG[:], rg2)
                b14 = wk.tile([P, FD], f32, tag="b14")
                nc.vector.tensor_tensor(b14[:], s3[:], s4[:], ALU.min)
                nc.vector.tensor_scalar_max(b14[:], b14[:], 0.0)

                def stt(dst, src, k, w):
                    nc.vector.scalar_tensor_tensor(
                        dst, src, float(k), w, ALU.mult, ALU.mult
                    )

                # level 2 blends
                u = wk.tile([P, FD], f32, tag="u")
                v = wk.tile([P, FD], f32, tag="v")
                b22 = wk.tile([P, FD], f32, tag="b22")
                stt(u[:], b13[:], rc2, wM[:])        # M * B1_3
                stt(v[:], b12[:], rc1, z[:])         # p * B1_2
                nc.vector.tensor_add(b22[:], u[:], v[:])

                u2 = wk.tile([P, FD], f32, tag="u2")
                v2 = wk.tile([P, FD], f32, tag="v2")
                b23 = wk.tile([P, FD], f32, tag="b23")
                stt(u2[:], b13[:], rc2, z[:])        # m * B1_3
                stt(v2[:], b14[:], rg1, wG[:])       # H * B1_4
                nc.vector.tensor_add(b23[:], u2[:], v2[:])

                u3 = wk.tile([P, FD], f32, tag="u3")
                v3 = wk.tile([P, FD], f32, tag="v3")
                b24 = wk.tile([P, FD], f32, tag="b24")
                stt(u3[:], b14[:], rg1, wh[:])       # h * B1_4
                stt(v3[:], b15[:], rg2, wG[:])       # E * B1_5
                nc.vector.tensor_add(b24[:], u3[:], v3[:])

                # level 3 -> interleaved output tile
                ot = io.tile([P, FD * 6], f32, tag="o")

                stt(ot[:, 0::6], b21[:], -rc1, wh[:])             # B3_0 = P*B2_1

                ua = wk.tile([P, FD], f32, tag="ua")
                va = wk.tile([P, FD], f32, tag="va")
                stt(ua[:], b22[:], rc2, wM[:])       # M * B2_2
                stt(va[:], b21[:], rc1, z[:])        # p * B2_1
                nc.vector.tensor_add(ot[:, 1::6], ua[:], va[:])

                ub = wk.tile([P, FD], f32, tag="ub")
                vb = wk.tile([P, FD], f32, tag="vb")
                stt(ub[:], b22[:], rc2, z[:])        # m * B2_2
                stt(vb[:], b23[:], 1.0, wG[:])       # G * B2_3
                nc.vector.tensor_add(ot[:, 2::6], ub[:], vb[:])

                uc = wk.tile([P, FD], f32, tag="uc")
                vc = wk.tile([P, FD], f32, tag="vc")
                nc.vector.tensor_mul(uc[:], z[:], b23[:])   # g * B2_3
                stt(vc[:], b24[:], rg1, wG[:])              # H * B2_4
                nc.vector.tensor_add(ot[:, 3::6], uc[:], vc[:])

                ud = wk.tile([P, FD], f32, tag="ud")
                vd = wk.tile([P, FD], f32, tag="vd")
                stt(ud[:], b24[:], rg1, wh[:])       # h * B2_4
                stt(vd[:], b25[:], rg2, wG[:])       # E * B2_5
                nc.vector.tensor_add(ot[:, 4::6], ud[:], vd[:])

                stt(ot[:, 5::6], b25[:], rg2, we[:])              # B3_5 = e*B2_5

                nc.sync.dma_start(o_ap[t], ot[:])

    nc.compile()
    return nc


def _get_compiled(knots):
    key = knots.tobytes()
    if key not in _cache:
        t = knots.astype(np.float64)
        ok = (
            knots.shape == (10,)
            and np.all(t[:4] == t[0])
            and np.all(t[6:] == t[9])
            and t[0] == 0.0
            and t[9] == 1.0
            and t[0] < t[4] < t[5] < t[9]
        )
        if not ok:
            _cache[key] = None
        else:
            _cache[key] = _build(float(t[4]), float(t[5]))
    return _cache[key]


def _reference_fallback(x, knots):
    """Numpy mirror of the jax reference, used only for unexpected knots."""
    t = knots.astype(np.float32)
    K = t.shape[0]
    xmin, xmax = x.min(), x.max()
    d = np.float32(np.float32(xmax - xmin) + np.float32(1e-8))
    z = ((x - xmin) / d).astype(np.float32)[:, None]
    left, right = t[None, :-1], t[None, 1:]
    B = ((z >= left) & (z < right)).astype(np.float32)
    B = np.where((z == t[-1]) & (right == t[-1]) & (left < right), np.float32(1.0), B)
    for dgr in range(1, 4):
        tL, tLd = t[: K - dgr - 1], t[dgr : K - 1]
        tR, tRd = t[1 : K - dgr], t[dgr + 1 : K]
        den1, den2 = tLd - tL, tRd - tR
        safe1 = np.where(den1 > 0, den1, 1.0).astype(np.float32)
        safe2 = np.where(den2 > 0, den2, 1.0).astype(np.float32)
        w1 = np.where(den1[None] > 0, (z - tL[None]) / safe1[None], 0.0).astype(np.float32)
        w2 = np.where(den2[None] > 0, (tRd[None] - z) / safe2[None], 0.0).astype(np.float32)
        B = (w1 * B[:, :-1] + w2 * B[:, 1:]).astype(np.float32)
    return B


def kernel(x, knots):
    from concourse import bass_utils

    x = np.ascontiguousarray(np.asarray(x, dtype=np.float32).ravel())
    knots = np.ascontiguousarray(np.asarray(knots, dtype=np.float32).ravel())
    assert x.shape[0] == N_POINTS, x.shape

    nc = _get_compiled(knots)
    if nc is None:  # unexpected knot structure: safe host fallback
        return _reference_fallback(x, knots)

    xmin = x.min()
    xmax = x.max()
    d = np.float32(np.float32(xmax - xmin) + np.float32(1e-8))
    s = np.float32(1.0) / d
    b = np.float32(-(xmin * s))
    c2f = np.float64(knots[5])
    stats = np.empty((P, 4), np.float32)
    stats[:, 0] = s
    stats[:, 1] = b
    stats[:, 2] = np.float32(-c2f / (1.0 - c2f))
    stats[:, 3] = 0.0

    shards = x.reshape(N_CORES, T_TILES, P, FD)
    in_maps = [{"x": shards[i], "stats": stats} for i in range(N_CORES)]
    res = bass_utils.run_bass_kernel_spmd(nc, in_maps, list(range(N_CORES)))
    out = np.empty((N_CORES, N_SHARD * 6), np.float32)
    for i in range(N_CORES):
        out[i] = res.results[i]["out"].reshape(-1)
    return out.reshape(N_POINTS, 6)


# revision 7
# speedup vs baseline: 1.0311x; 1.0311x over previous
"""Trainium2 Bass kernel for clamped cubic B-spline basis evaluation.

Computes, for x: [N] f32 and a clamped knot vector t (K=10, degree 3):
    z = (x - min(x)) / (max(x) - min(x) + 1e-8)
    out[n, j] = B_j^3(z[n]),  j = 0..5   -> [N, 6] f32

Strategy: trivially data-parallel over 8 NeuronCores (N/8 points each).
Per core, points stream through [128 x FD] tiles.  The Cox-de Boor
recursion is evaluated with a continuous reformulation (relu/min hats,
complementary-weight blends); all selection boundaries are continuous
crossings so the masked reference is matched to ~1e-6 without branches.

Work is spread across four engines:
  - ACT: normalization, the two corner relu chains + squares/cubes
  - DVE: five fused custom ops (hats, one fused blend, ramp-blend,
    E*B2_5 corner) + the strided output-assembly adds
  - GPSIMD: scalar_tensor_tensor products/partial sums
  - PE: unused (fp32 matmul is slow on TRN2)

The [N, 6] output is assembled interleaved in SBUF (stride-6 writes by
the final ops) so each DRAM store is a single contiguous DMA.
"""

import numpy as np

N_POINTS = 8_388_608
N_CORES = 8
P = 128          # SBUF partitions
FD = 512         # free-dim elements per tile
N_SHARD = N_POINTS // N_CORES
TILE_ELEMS = P * FD
T_TILES = N_SHARD // TILE_ELEMS

_cache = {}
_ops = None


def _register_ops():
    """Register the fused custom DVE ops (idempotent)."""
    global _ops
    if _ops is not None:
        return _ops
    import concourse.dve_ops as D
    from concourse.dve_spec import Spec, Src0, Src1, C0, C1, C2, One, relu, sq, minn, lower
    from concourse.dve_uop import DveOpSpec

    def reg(name, body):
        if name in D._SUB_OPCODE_FOR_NAME:
            return next(o for o in D.OPS if o.name == name)
        spec = Spec(body=body)
        row = 1 + len(D.OPS)
        assert row < 0x20, "custom-DVE opcode rows exhausted"
        shas = {}
        for ver in ("v3", "v4"):
            tmp = DveOpSpec(
                name=name, opcode=row, uops=lower(spec, ver=ver),
                rd1_en=D.has_src1(spec),
            )
            shas[ver] = tmp.sha(ver)
        op = D.DveOp(name, spec, False, uops_sha=shas)
        D.OPS.append(op)
        D._SUB_OPCODE_FOR_NAME[name] = row
        D.CUSTOM_DVE_SPECS[name] = spec
        return op

    e = Src0 * C0 + C1
    p = Src0 * C0
    _ops = {
        # relu(min(z*c0, z*c1 + c2))                      -> B1_3
        "BSPL_HAT_A": reg("BSPL_HAT_A", relu(minn(Src0 * C0, Src0 * C1 + C2))),
        # relu(min(z*c0 + c1, (1-z)*c2))                  -> B1_4 (scaled)
        "BSPL_HAT_B": reg("BSPL_HAT_B", relu(minn(Src0 * C0 + C1, (One - Src0) * C2))),
        # relu(z*c0 + c1)^3                               -> B3_0 / B3_5
        "BSPL_CUBE": reg("BSPL_CUBE", (lambda t: sq(t) * t)(relu(e))),
        # (1-e)*relu(e)^2 = E*B2_5                        -> OUT4 partial
        "BSPL_ECORN": reg("BSPL_ECORN", (One - e) * sq(relu(e))),
        # p*relu(1-p)^2 = p*B2_1                          -> OUT1 partial
        "BSPL_PCORN": reg("BSPL_PCORN", p * sq(relu(One - p))),
        # relu(e - e^2) + (z - c2)*Src1 = E*B1_5 + h*B1_4 -> B2_4
        "BSPL_ADDRAMP": reg("BSPL_ADDRAMP", relu(e - sq(e)) + (Src0 - C2) * Src1),
        # p*relu(1-p) + (1 - z*c1)*Src1 = p*B1_2 + M*B1_3 -> B2_2
        "BSPL_BLEND2": reg("BSPL_BLEND2", p * relu(One - p) + (One - Src0 * C1) * Src1),
    }
    return _ops


def _build(c1, c2, act_cube=True):
    """Build + compile the per-core Bass program. c1, c2: interior knots."""
    import concourse.bacc as bacc
    import concourse.mybir as mybir
    import concourse.tile as tile

    ops = _register_ops()
    f32 = mybir.dt.float32
    AF = mybir.ActivationFunctionType
    ALU = mybir.AluOpType

    nc = bacc.Bacc("TRN2", target_bir_lowering=False, debug=False)
    x_d = nc.dram_tensor("x", [T_TILES, P, FD], f32, kind="ExternalInput")
    st_d = nc.dram_tensor("stats", [P, 4], f32, kind="ExternalInput")
    o_d = nc.dram_tensor("out", [T_TILES, P, FD * 6], f32, kind="ExternalOutput")
    x_ap, st_ap, o_ap = x_d.ap(), st_d.ap(), o_d.ap()

    rc1 = 1.0 / c1
    rc2 = 1.0 / c2
    rdc = 1.0 / (c2 - c1)
    rg1 = 1.0 / (1.0 - c1)
    rg2 = 1.0 / (1.0 - c2)

    def cust(op, out, in0, s0=0.0, s1=0.0, imm2=0.0, in1=None):
        nc.vector._custom_dve(ops[op], out=out, in0=in0, in1=in1,
                              s0=s0, s1=s1, imm2=imm2)

    with tile.TileContext(nc) as tc:
        with (
            tc.tile_pool(name="io", bufs=3) as io,
            tc.tile_pool(name="wk", bufs=2) as wk,
            tc.tile_pool(name="cst", bufs=1) as cst,
        ):
            st = cst.tile([P, 4], f32, tag="st", name="st")
            nc.sync.dma_start(st[:], st_ap[:])
            s_ap = st[:, 0:1]
            b_ap = st[:, 1:2]
            b15_ap = st[:, 2:3]

            def wt(tag):
                return wk.tile([P, FD], f32, tag=tag, name=tag)

            for t in range(T_TILES):
                xt = io.tile([P, FD], f32, tag="x", name="x")
                nc.sync.dma_start(xt[:], x_ap[t])

                # ---- ACT: normalize + corner chains -------------------
                z = wt("z")
                # z >= 0 by construction, so Relu == affine here (Copy
                # does not accept an AP bias).
                nc.scalar.activation(z[:], xt[:], AF.Relu, bias=b_ap, scale=s_ap)
                b12 = wt("b12")
                nc.scalar.activation(b12[:], z[:], AF.Relu, bias=1.0, scale=-rc1)
                b15 = wt("b15")
                nc.scalar.activation(b15[:], z[:], AF.Relu, bias=b15_ap, scale=rg2)

                # ---- DVE customs: hats + fused blends/corners ---------
                b13 = wt("b13")
                cust("BSPL_HAT_A", b13[:], z[:], rc1, -rdc, c2 * rdc)
                b14s = wt("b14s")   # rg1 * B1_4
                cust("BSPL_HAT_B", b14s[:], z[:], rdc * rg1, -c1 * rdc * rg1, rg2 * rg1)
                b22 = wt("b22")     # B2_2 = p*B1_2 + M*B1_3
                cust("BSPL_BLEND2", b22[:], z[:], rc1, rc2, in1=b13[:])
                b24 = wt("b24")     # B2_4 = E*B1_5 + h*B1_4
                cust("BSPL_ADDRAMP", b24[:], z[:], rg2, -c2 * rg2, c1, in1=b14s[:])
                pc = wt("pc")       # p * B2_1
                cust("BSPL_PCORN", pc[:], z[:], rc1)
                ec = wt("ec")       # E * B2_5
                cust("BSPL_ECORN", ec[:], z[:], rg2, -c2 * rg2)

                # ---- products + combines (DVE STT / GPSIMD TT / DMA) --
                zb13s = wt("zb13s")  # m * B1_3
                nc.vector.scalar_tensor_tensor(zb13s[:], z[:], rc2, b13[:], ALU.mult, ALU.mult)
                mz22n = wt("mz22n")  # -m * B2_2
                nc.vector.scalar_tensor_tensor(mz22n[:], z[:], -rc2, b22[:], ALU.mult, ALU.mult)

                zb14 = wt("zb14")    # z * b14s
                nc.gpsimd.tensor_tensor(zb14[:], z[:], b14s[:], ALU.mult)
                t23 = wt("t23")      # H*B1_4 = b14s - z*b14s
                nc.gpsimd.tensor_tensor(t23[:], b14s[:], zb14[:], ALU.subtract)
                # t23 += m*B1_3  ->  B2_3   (DMA inline-add)
                nc.gpsimd.dma_start(t23[:], zb13s[:], accum_op=ALU.add)
                b23 = t23

                zb23 = wt("zb23")    # z * B2_3
                nc.gpsimd.tensor_tensor(zb23[:], z[:], b23[:], ALU.mult)
                t2 = wt("t2")        # G*B2_3 = B2_3 - z*B2_3
                nc.gpsimd.tensor_tensor(t2[:], b23[:], zb23[:], ALU.subtract)
                zb24 = wt("zb24")    # z * B2_4
                nc.gpsimd.tensor_tensor(zb24[:], z[:], b24[:], ALU.mult)

                # ---- output assembly (interleaved SBUF tile) ----------
                ot = io.tile([P, FD * 6], f32, tag="o", name="o")

                ln2 = wt("ln2")
                nc.scalar.activation(ln2[:], b12[:], AF.Ln)
                nc.scalar.activation(ot[:, 0::6], ln2[:], AF.Exp, scale=3.0)
                ln5 = wt("ln5")
                nc.scalar.activation(ln5[:], b15[:], AF.Ln)
                nc.scalar.activation(ot[:, 5::6], ln5[:], AF.Exp, scale=3.0)

                # OUT2 = m*B2_2 + G*B2_3   (reads mz22n before the accum below)
                nc.vector.scalar_tensor_tensor(ot[:, 2::6], mz22n[:], -1.0, t2[:], ALU.mult, ALU.add)
                # mz22n += B2_2  ->  M*B2_2   (DMA inline-add)
                nc.gpsimd.dma_start(mz22n[:], b22[:], accum_op=ALU.add)
                # OUT1 = p*B2_1 + M*B2_2
                nc.gpsimd.tensor_tensor(ot[:, 1::6], pc[:], mz22n[:], ALU.add)

                t3 = wt("t3")        # (1-z) * B2_4
                nc.vector.scalar_tensor_tensor(t3[:], zb24[:], -1.0, b24[:], ALU.mult, ALU.add)
                # OUT3 = z*B2_3 + H*B2_4
                nc.vector.scalar_tensor_tensor(ot[:, 3::6], t3[:], rg1, zb23[:], ALU.mult, ALU.add)
                t4 = wt("t4")        # E*B2_5 - c1*rg1*B2_4
                nc.vector.scalar_tensor_tensor(t4[:], b24[:], -c1 * rg1, ec[:], ALU.mult, ALU.add)
                # OUT4 = h*B2_4 + E*B2_5
                nc.vector.scalar_tensor_tensor(ot[:, 4::6], zb24[:], rg1, t4[:], ALU.mult, ALU.add)

                nc.sync.dma_start(o_ap[t], ot[:])

    nc.compile()
    return nc


def _get_compiled(knots):
    key = knots.tobytes()
    if key not in _cache:
        t = knots.astype(np.float64)
        ok = (
            knots.shape == (10,)
            and np.all(t[:4] == t[0])
            and np.all(t[6:] == t[9])
            and t[0] == 0.0
            and t[9] == 1.0
            and t[0] < t[4] < t[5] < t[9]
        )
        if not ok:
            _cache[key] = None
        else:
            _cache[key] = _build(float(t[4]), float(t[5]))
    return _cache[key]


def _reference_fallback(x, knots):
    """Numpy mirror of the jax reference, used only for unexpected knots."""
    t = knots.astype(np.float32)
    K = t.shape[0]
    xmin, xmax = x.min(), x.max()
    d = np.float32(np.float32(xmax - xmin) + np.float32(1e-8))
    z = ((x - xmin) / d).astype(np.float32)[:, None]
    left, right = t[None, :-1], t[None, 1:]
    B = ((z >= left) & (z < right)).astype(np.float32)
    B = np.where((z == t[-1]) & (right == t[-1]) & (left < right), np.float32(1.0), B)
    for dgr in range(1, 4):
        tL, tLd = t[: K - dgr - 1], t[dgr : K - 1]
        tR, tRd = t[1 : K - dgr], t[dgr + 1 : K]
        den1, den2 = tLd - tL, tRd - tR
        safe1 = np.where(den1 > 0, den1, 1.0).astype(np.float32)
        safe2 = np.where(den2 > 0, den2, 1.0).astype(np.float32)
        w1 = np.where(den1[None] > 0, (z - tL[None]) / safe1[None], 0.0).astype(np.float32)
        w2 = np.where(den2[None] > 0, (tRd[None] - z) / safe2[None], 0.0).astype(np.float32)
        B = (w1 * B[:, :-1] + w2 * B[:, 1:]).astype(np.float32)
    return B


def kernel(x, knots):
    from concourse import bass_utils

    x = np.ascontiguousarray(np.asarray(x, dtype=np.float32).ravel())
    knots = np.ascontiguousarray(np.asarray(knots, dtype=np.float32).ravel())
    assert x.shape[0] == N_POINTS, x.shape

    nc = _get_compiled(knots)
    if nc is None:  # unexpected knot structure: safe host fallback
        return _reference_fallback(x, knots)

    xmin = x.min()
    xmax = x.max()
    d = np.float32(np.float32(xmax - xmin) + np.float32(1e-8))
    s = np.float32(1.0) / d
    b = np.float32(-(xmin * s))
    c2f = np.float64(knots[5])
    stats = np.empty((P, 4), np.float32)
    stats[:, 0] = s
    stats[:, 1] = b
    stats[:, 2] = np.float32(-c2f / (1.0 - c2f))
    stats[:, 3] = 0.0

    shards = x.reshape(N_CORES, T_TILES, P, FD)
    in_maps = [{"x": shards[i], "stats": stats} for i in range(N_CORES)]
    res = bass_utils.run_bass_kernel_spmd(nc, in_maps, list(range(N_CORES)))
    out = np.empty((N_CORES, N_SHARD * 6), np.float32)
    for i in range(N_CORES):
        out[i] = res.results[i]["out"].reshape(-1)
    return out.reshape(N_POINTS, 6)


# revision 8
# speedup vs baseline: 1.5376x; 1.4912x over previous
"""Trainium2 Bass kernel for clamped cubic B-spline basis evaluation.

Computes, for x: [N] f32 and a clamped knot vector t (K=10, degree 3):
    z = (x - min(x)) / (max(x) - min(x) + 1e-8)
    out[n, j] = B_j^3(z[n]),  j = 0..5   -> [N, 6] f32

Strategy: trivially data-parallel over 8 NeuronCores (N/8 points each).
Per core, points stream through [128 x FD] tiles.  The Cox-de Boor
recursion is evaluated with a continuous reformulation (relu/min hats,
complementary-weight blends); all selection boundaries are continuous
crossings so the masked reference is matched to ~1e-6 without branches.

Work is spread across four engines:
  - ACT: normalization, the two corner relu chains + squares/cubes
  - DVE: five fused custom ops (hats, one fused blend, ramp-blend,
    E*B2_5 corner) + the strided output-assembly adds
  - GPSIMD: scalar_tensor_tensor products/partial sums
  - PE: unused (fp32 matmul is slow on TRN2)

The [N, 6] output is assembled interleaved in SBUF (stride-6 writes by
the final ops) so each DRAM store is a single contiguous DMA.
"""

import numpy as np

N_POINTS = 8_388_608
N_CORES = 8
P = 128          # SBUF partitions
FD = 512         # free-dim elements per tile
N_SHARD = N_POINTS // N_CORES
TILE_ELEMS = P * FD
T_TILES = N_SHARD // TILE_ELEMS

_cache = {}
_ops = None


def _register_ops():
    """Register the fused custom DVE ops (idempotent)."""
    global _ops
    if _ops is not None:
        return _ops
    import concourse.dve_ops as D
    from concourse.dve_spec import Spec, Src0, Src1, C0, C1, C2, One, relu, sq, minn, lower
    from concourse.dve_uop import DveOpSpec

    def reg(name, body):
        if name in D._SUB_OPCODE_FOR_NAME:
            return next(o for o in D.OPS if o.name == name)
        spec = Spec(body=body)
        row = 1 + len(D.OPS)
        assert row < 0x20, "custom-DVE opcode rows exhausted"
        shas = {}
        for ver in ("v3", "v4"):
            tmp = DveOpSpec(
                name=name, opcode=row, uops=lower(spec, ver=ver),
                rd1_en=D.has_src1(spec),
            )
            shas[ver] = tmp.sha(ver)
        op = D.DveOp(name, spec, False, uops_sha=shas)
        D.OPS.append(op)
        D._SUB_OPCODE_FOR_NAME[name] = row
        D.CUSTOM_DVE_SPECS[name] = spec
        return op

    e = Src0 * C0 + C1
    p = Src0 * C0
    _ops = {
        # relu(min(z*c0, z*c1 + c2))                      -> B1_3
        "BSPL_HAT_A": reg("BSPL_HAT_A", relu(minn(Src0 * C0, Src0 * C1 + C2))),
        # relu(min(z*c0 + c1, (1-z)*c2))                  -> B1_4 (scaled)
        "BSPL_HAT_B": reg("BSPL_HAT_B", relu(minn(Src0 * C0 + C1, (One - Src0) * C2))),
        # relu(z*c0 + c1)^3                               -> B3_0 / B3_5
        "BSPL_CUBE": reg("BSPL_CUBE", (lambda t: sq(t) * t)(relu(e))),
        # (1-e)*relu(e)^2 = E*B2_5                        -> OUT4 partial
        "BSPL_ECORN": reg("BSPL_ECORN", (One - e) * sq(relu(e))),
        # p*relu(1-p)^2 = p*B2_1                          -> OUT1 partial
        "BSPL_PCORN": reg("BSPL_PCORN", p * sq(relu(One - p))),
        # relu(e - e^2) + (z - c2)*Src1 = E*B1_5 + h*B1_4 -> B2_4
        "BSPL_ADDRAMP": reg("BSPL_ADDRAMP", relu(e - sq(e)) + (Src0 - C2) * Src1),
        # p*relu(1-p) + (1 - z*c1)*Src1 = p*B1_2 + M*B1_3 -> B2_2
        "BSPL_BLEND2": reg("BSPL_BLEND2", p * relu(One - p) + (One - Src0 * C1) * Src1),
    }
    return _ops


def _build(c1, c2, act_cube=True):
    """Build + compile the per-core Bass program. c1, c2: interior knots."""
    import concourse.bacc as bacc
    import concourse.mybir as mybir
    import concourse.tile as tile

    ops = _register_ops()
    f32 = mybir.dt.float32
    AF = mybir.ActivationFunctionType
    ALU = mybir.AluOpType

    nc = bacc.Bacc("TRN2", target_bir_lowering=False, debug=False)
    x_d = nc.dram_tensor("x", [T_TILES, P, FD], f32, kind="ExternalInput")
    st_d = nc.dram_tensor("stats", [P, 4], f32, kind="ExternalInput")
    o_d = nc.dram_tensor("out", [T_TILES, P, FD * 6], f32, kind="ExternalOutput")
    x_ap, st_ap, o_ap = x_d.ap(), st_d.ap(), o_d.ap()

    rc1 = 1.0 / c1
    rc2 = 1.0 / c2
    rdc = 1.0 / (c2 - c1)
    rg1 = 1.0 / (1.0 - c1)
    rg2 = 1.0 / (1.0 - c2)

    def cust(op, out, in0, s0=0.0, s1=0.0, imm2=0.0, in1=None):
        nc.vector._custom_dve(ops[op], out=out, in0=in0, in1=in1,
                              s0=s0, s1=s1, imm2=imm2)

    with tile.TileContext(nc) as tc:
        with (
            tc.tile_pool(name="io", bufs=3) as io,
            tc.tile_pool(name="wk", bufs=3) as wk,
            tc.tile_pool(name="cst", bufs=1) as cst,
        ):
            st = cst.tile([P, 4], f32, tag="st", name="st")
            nc.sync.dma_start(st[:], st_ap[:])
            s_ap = st[:, 0:1]
            b_ap = st[:, 1:2]
            b15_ap = st[:, 2:3]

            def wt(tag):
                return wk.tile([P, FD], f32, tag=tag, name=tag)

            for t in range(T_TILES):
                xt = io.tile([P, FD], f32, tag="x", name="x")
                nc.sync.dma_start(xt[:], x_ap[t])

                # ---- ACT: normalize + corner chains -------------------
                z = wt("z")
                # z >= 0 by construction, so Relu == affine here (Copy
                # does not accept an AP bias).
                nc.scalar.activation(z[:], xt[:], AF.Relu, bias=b_ap, scale=s_ap)
                b12 = wt("b12")
                nc.scalar.activation(b12[:], z[:], AF.Relu, bias=1.0, scale=-rc1)
                b15 = wt("b15")
                nc.scalar.activation(b15[:], z[:], AF.Relu, bias=b15_ap, scale=rg2)

                # ---- DVE customs: hats + fused blends/corners ---------
                b13 = wt("b13")
                cust("BSPL_HAT_A", b13[:], z[:], rc1, -rdc, c2 * rdc)
                b14s = wt("b14s")   # rg1 * B1_4
                cust("BSPL_HAT_B", b14s[:], z[:], rdc * rg1, -c1 * rdc * rg1, rg2 * rg1)
                b22 = wt("b22")     # B2_2 = p*B1_2 + M*B1_3
                cust("BSPL_BLEND2", b22[:], z[:], rc1, rc2, in1=b13[:])
                b24 = wt("b24")     # B2_4 = E*B1_5 + h*B1_4
                cust("BSPL_ADDRAMP", b24[:], z[:], rg2, -c2 * rg2, c1, in1=b14s[:])
                pc = wt("pc")       # p * B2_1
                cust("BSPL_PCORN", pc[:], z[:], rc1)
                ec = wt("ec")       # E * B2_5
                cust("BSPL_ECORN", ec[:], z[:], rg2, -c2 * rg2)

                # ---- products + combines (DVE STT / GPSIMD TT / DMA) --
                zb13s = wt("zb13s")  # m * B1_3
                nc.vector.scalar_tensor_tensor(zb13s[:], z[:], rc2, b13[:], ALU.mult, ALU.mult)
                mz22n = wt("mz22n")  # -m * B2_2
                nc.vector.scalar_tensor_tensor(mz22n[:], z[:], -rc2, b22[:], ALU.mult, ALU.mult)

                zb14 = wt("zb14")    # z * b14s
                nc.gpsimd.tensor_tensor(zb14[:], z[:], b14s[:], ALU.mult)
                t23 = wt("t23")      # H*B1_4 = b14s - z*b14s
                nc.gpsimd.tensor_tensor(t23[:], b14s[:], zb14[:], ALU.subtract)
                b23 = wt("b23")      # B2_3 = m*B1_3 + H*B1_4
                nc.vector.scalar_tensor_tensor(b23[:], zb13s[:], 1.0, t23[:], ALU.mult, ALU.add)

                zb23 = wt("zb23")    # z * B2_3
                nc.gpsimd.tensor_tensor(zb23[:], z[:], b23[:], ALU.mult)
                t2 = wt("t2")        # G*B2_3 = B2_3 - z*B2_3
                nc.gpsimd.tensor_tensor(t2[:], b23[:], zb23[:], ALU.subtract)
                zb24 = wt("zb24")    # z * B2_4
                nc.gpsimd.tensor_tensor(zb24[:], z[:], b24[:], ALU.mult)

                # ---- output assembly (interleaved SBUF tile) ----------
                ot = io.tile([P, FD * 6], f32, tag="o", name="o")

                ln2 = wt("ln2")
                nc.scalar.activation(ln2[:], b12[:], AF.Ln)
                nc.scalar.activation(ot[:, 0::6], ln2[:], AF.Exp, scale=3.0)
                ln5 = wt("ln5")
                nc.scalar.activation(ln5[:], b15[:], AF.Ln)
                nc.scalar.activation(ot[:, 5::6], ln5[:], AF.Exp, scale=3.0)

                # OUT2 = m*B2_2 + G*B2_3
                nc.vector.scalar_tensor_tensor(ot[:, 2::6], mz22n[:], -1.0, t2[:], ALU.mult, ALU.add)
                t1 = wt("t1")        # M*B2_2 = B2_2 - m*B2_2
                nc.gpsimd.tensor_tensor(t1[:], b22[:], mz22n[:], ALU.add)
                # OUT1 = p*B2_1 + M*B2_2
                nc.gpsimd.tensor_tensor(ot[:, 1::6], pc[:], t1[:], ALU.add)

                t3 = wt("t3")        # (1-z) * B2_4
                nc.vector.scalar_tensor_tensor(t3[:], zb24[:], -1.0, b24[:], ALU.mult, ALU.add)
                # OUT3 = z*B2_3 + H*B2_4
                nc.vector.scalar_tensor_tensor(ot[:, 3::6], t3[:], rg1, zb23[:], ALU.mult, ALU.add)
                t4 = wt("t4")        # E*B2_5 - c1*rg1*B2_4
                nc.vector.scalar_tensor_tensor(t4[:], b24[:], -c1 * rg1, ec[:], ALU.mult, ALU.add)
                # OUT4 = h*B2_4 + E*B2_5
                nc.vector.scalar_tensor_tensor(ot[:, 4::6], zb24[:], rg1, t4[:], ALU.mult, ALU.add)

                nc.sync.dma_start(o_ap[t], ot[:])

    # Force every activation onto the one table set that covers
    # relu/ln/exp/square, so the table is loaded once instead of
    # thrashing between per-function sets (~2.7us per switch).
    import concourse.hw_specs as hw_specs
    import concourse.bacc as bacc_mod
    _orig_gat = hw_specs.get_activation_tables
    _one = "natural_log_exp_and_others"

    def _gat(arch):
        t = _orig_gat(arch)
        assert _one in t
        return {k: (v if k == _one else set()) for k, v in t.items()}

    hw_specs.get_activation_tables = _gat
    bacc_patch = getattr(bacc_mod, "get_activation_tables", None)
    if bacc_patch is not None:
        bacc_mod.get_activation_tables = _gat
    try:
        nc.compile()
    finally:
        hw_specs.get_activation_tables = _orig_gat
        if bacc_patch is not None:
            bacc_mod.get_activation_tables = bacc_patch
    return nc


def _get_compiled(knots):
    key = knots.tobytes()
    if key not in _cache:
        t = knots.astype(np.float64)
        ok = (
            knots.shape == (10,)
            and np.all(t[:4] == t[0])
            and np.all(t[6:] == t[9])
            and t[0] == 0.0
            and t[9] == 1.0
            and t[0] < t[4] < t[5] < t[9]
        )
        if not ok:
            _cache[key] = None
        else:
            _cache[key] = _build(float(t[4]), float(t[5]))
    return _cache[key]


def _reference_fallback(x, knots):
    """Numpy mirror of the jax reference, used only for unexpected knots."""
    t = knots.astype(np.float32)
    K = t.shape[0]
    xmin, xmax = x.min(), x.max()
    d = np.float32(np.float32(xmax - xmin) + np.float32(1e-8))
    z = ((x - xmin) / d).astype(np.float32)[:, None]
    left, right = t[None, :-1], t[None, 1:]
    B = ((z >= left) & (z < right)).astype(np.float32)
    B = np.where((z == t[-1]) & (right == t[-1]) & (left < right), np.float32(1.0), B)
    for dgr in range(1, 4):
        tL, tLd = t[: K - dgr - 1], t[dgr : K - 1]
        tR, tRd = t[1 : K - dgr], t[dgr + 1 : K]
        den1, den2 = tLd - tL, tRd - tR
        safe1 = np.where(den1 > 0, den1, 1.0).astype(np.float32)
        safe2 = np.where(den2 > 0, den2, 1.0).astype(np.float32)
        w1 = np.where(den1[None] > 0, (z - tL[None]) / safe1[None], 0.0).astype(np.float32)
        w2 = np.where(den2[None] > 0, (tRd[None] - z) / safe2[None], 0.0).astype(np.float32)
        B = (w1 * B[:, :-1] + w2 * B[:, 1:]).astype(np.float32)
    return B


def kernel(x, knots):
    from concourse import bass_utils

    x = np.ascontiguousarray(np.asarray(x, dtype=np.float32).ravel())
    knots = np.ascontiguousarray(np.asarray(knots, dtype=np.float32).ravel())
    assert x.shape[0] == N_POINTS, x.shape

    nc = _get_compiled(knots)
    if nc is None:  # unexpected knot structure: safe host fallback
        return _reference_fallback(x, knots)

    xmin = x.min()
    xmax = x.max()
    d = np.float32(np.float32(xmax - xmin) + np.float32(1e-8))
    s = np.float32(1.0) / d
    b = np.float32(-(xmin * s))
    c2f = np.float64(knots[5])
    stats = np.empty((P, 4), np.float32)
    stats[:, 0] = s
    stats[:, 1] = b
    stats[:, 2] = np.float32(-c2f / (1.0 - c2f))
    stats[:, 3] = 0.0

    shards = x.reshape(N_CORES, T_TILES, P, FD)
    in_maps = [{"x": shards[i], "stats": stats} for i in range(N_CORES)]
    res = bass_utils.run_bass_kernel_spmd(nc, in_maps, list(range(N_CORES)))
    out = np.empty((N_CORES, N_SHARD * 6), np.float32)
    for i in range(N_CORES):
        out[i] = res.results[i]["out"].reshape(-1)
    return out.reshape(N_POINTS, 6)
